# Initial kernel scaffold
#
"""Trainium2 Bass kernel for nn_DecoderRNN (LSTM decoder + vocab projection).

Sharding: batch 128 -> 16 per core across 8 cores (data parallel). LSTM
weights/embedding/fc replicated. All matmuls in bf16 (fp32 PSUM accumulate).

Per-core layout: recurrent state kept transposed (D on partitions, batch on
free dim) so the h @ Whh^T matmuls chain without per-step transposes:
  out gates^T[m-chunk 128, b 16] = WhhT_tile[k 128, m 128]^T @ hT[k 128, b 16]
Gate order host-permuted to [i, f, o, g] so sigmoid runs on one contiguous
block. Input-side gate preactivations (x_t @ Wih^T + biases) are precomputed
for all timesteps in two batched matmuls and spilled to DRAM, streamed back
per step. Logits computed per 128-token (t,b) chunk with H^T tiles stationary
and fc_W^T streaming; log-softmax without max-subtraction (logits are small:
h is layernormed), exp+sum via ACT accum_out.
"""

import numpy as np
import ml_dtypes

import concourse.bass as bass
import concourse.mybir as mybir
import concourse.tile as tile
from concourse import bacc
from concourse.masks import make_identity

F32 = mybir.dt.float32
BF16 = mybir.dt.bfloat16
I32 = mybir.dt.int32
AF = mybir.ActivationFunctionType
OP = mybir.AluOpType

NC = 8          # cores
B = 128         # global batch
BC = B // NC    # batch per core (16)
T = 33          # timesteps (1 feature + 32 caption)
LCAP = 32
D = 512
V = 10000
G = 4 * D       # 2048 gate dim
NK = D // 128   # 4 k-chunks
NM = G // 128   # 16 m-chunks
TOK = BC * T    # 528 tokens per core
NBT = BC * LCAP // 128  # 4 output (t,b) chunks of 128
NW = 20         # vocab chunks of 500
VW = V // NW    # 500
LN_EPS = 1e-5

bf16 = ml_dtypes.bfloat16


def _ap_insert_bcast(ap_obj, pos, count):
    """Insert a stride-0 dim of size `count` at free-dim position `pos`
    (0 = outermost free dim)."""
    dims = [list(d) for d in ap_obj.ap]
    dims.insert(1 + pos, [0, count])
    return bass.AP(ap_obj.tensor, ap_obj.offset, dims)


def _emit(nc, tc, d, flags):
    """Emit the full per-core program. d: dict of dram APs."""
    has_fcb = flags["has_fcb"]
    has_ln = flags["has_ln"]

    ctx_pools = []

    cpool = tc.tile_pool(name="const", bufs=1).__enter__()
    ppool = tc.tile_pool(name="persist", bufs=1).__enter__()
    dpool = tc.tile_pool(name="dram", bufs=1, space="DRAM").__enter__()
    ctx_pools += [cpool, ppool, dpool]

    ident = cpool.tile([128, 128], BF16)
    make_identity(nc, ident[:])
    ones_col = cpool.tile([128, 1], F32)   # lhsT for partition sums
    nc.vector.memset(ones_col[:], 1.0)
    ones_row = cpool.tile([1, 128], F32)   # lhsT for broadcast matmul
    nc.vector.memset(ones_row[:], 1.0)

    # ---- persistent tiles ----
    whh_sb = ppool.tile([128, 2 * NK * G], BF16)   # (l, k, g) 32KB/part
    for l in range(2):
        nc.sync.dma_start(
            whh_sb[:, l * NK * G:(l + 1) * NK * G],
            d["whhT"][l].rearrange("(k p) g -> p (k g)", p=128))
    bias_sb = ppool.tile([128, 32], F32)           # (l, m)
    nc.sync.dma_start(bias_sb[:], d["biasT"].ap())
    fc_sb = ppool.tile([128, NK * V], BF16)        # (k, v) 80KB/part
    nc.sync.dma_start(fc_sb[:], d["fcWT"].ap().rearrange("(k p) v -> p (k v)", p=128))
    if has_fcb:
        fcb_sb = ppool.tile([1, V], BF16)
        nc.sync.dma_start(fcb_sb[:], d["fcb"].ap().rearrange("v -> 1 v"))
        ones_row16 = cpool.tile([1, 128], BF16)
        nc.vector.memset(ones_row16[:], 1.0)
    if has_ln:
        lng_sb = ppool.tile([128, NK], F32)
        nc.sync.dma_start(lng_sb[:], d["lng"].ap().rearrange("(c p) -> p c", p=128))
        lnb_sb = ppool.tile([128, NK], F32)
        nc.sync.dma_start(lnb_sb[:], d["lnb"].ap().rearrange("(c p) -> p c", p=128))

    HT = ppool.tile([128, NK * BC * LCAP], BF16)   # (k, bt) bt=(t-1)*16+b
    xT = ppool.tile([128, NK * TOK], BF16)         # (k, tok) tok=b*33+t
    hT = ppool.tile([128, NK * BC], BF16)          # state (k, b)
    h1T = ppool.tile([128, NK * BC], BF16)
    cT = ppool.tile([128, NK * BC], F32)
    nc.vector.memset(hT[:], 0.0)
    nc.vector.memset(cT[:], 0.0)

    preT_d = dpool.tile([2, TOK, NM, 128], F32)    # DRAM spill of pre-gates

    # ---- prologue: embedding gather + transpose into xT ----
    lpool = tc.tile_pool(name="prolog", bufs=2).__enter__()
    lps = tc.tile_pool(name="prolog_ps", bufs=2, space="PSUM").__enter__()
    ctx_pools += [lpool, lps]

    cap_sb = lpool.tile([128, 4], I32, tag="cap")
    nc.sync.dma_start(cap_sb[:], d["cap"].ap().rearrange("(g p) -> p g", p=128))

    # features -> xT slots t=0
    xT4 = xT[:].rearrange("p (k b s) -> p k b s", k=NK, b=BC)  # s = t 0..32
    nc.sync.dma_start(
        xT4[:, :, :, 0:1],
        d["featT"].ap().rearrange("(k p) b -> p k b", p=128))

    for g in range(4):
        gt = lpool.tile([128, D], BF16, tag="gather")
        nc.gpsimd.indirect_dma_start(
            out=gt[:], out_offset=None,
            in_=d["embW"].ap(),
            in_offset=bass.IndirectOffsetOnAxis(ap=cap_sb[:, g:g + 1], axis=0))
        for k in range(NK):
            tps = lps.tile([128, 128], F32, tag="tps")
            nc.tensor.transpose(tps[:], gt[:, k * 128:(k + 1) * 128], ident[:])
            # tokens e = g*128 + r*32 + s -> b = 4g+r, slot = b*33 + 1 + s
            nc.vector.tensor_copy(
                xT4[:, k, 4 * g:4 * g + 4, 1:33],
                tps[:].rearrange("p (r s) -> p r s", r=4))

    # ---- pre-gate precompute jobs: (l, m, t-half) ----
    wih_tiles = {}
    for l in range(2):
        wt = lpool.tile([128, NK * G], BF16, tag="wih")
        nc.sync.dma_start(
            wt[:], d["wihT"][l].rearrange("(k p) g -> p (k g)", p=128))
        wih_tiles[l] = wt

    t_halves = [(0, 17), (17, 16)]  # (t0, nt)

    def emit_pre_job(l, m, hh):
        t0, nt = t_halves[hh]
        n = BC * nt
        pps = lps.tile([128, 272], F32, tag="pps")
        for k in range(NK):
            # rhs: xT[:, k, b, t0:t0+nt] strided (b,t)
            rhs = xT4[:, k, :, t0:t0 + nt]
            nc.tensor.matmul(
                pps[:, :n],
                wih_tiles[l][:, (k * G + m * 128):(k * G + m * 128 + 128)],
                rhs, start=(k == 0), stop=(k == NK - 1))
        sc = lpool.tile([128, 272], F32, tag="psc")
        nc.scalar.activation(sc[:, :n], pps[:, :n], AF.Identity,
                             bias=bias_sb[:, l * NM + m:l * NM + m + 1])
        # DRAM (l, tok=b*33+t, m, p); psum iter (b, t)
        dst = preT_d[l, :, m, :].rearrange("(b s) p -> p b s", b=BC)
        nc.sync.dma_start(dst[:, :, t0:t0 + nt],
                          sc[:, :n].rearrange("p (b s) -> p b s", b=BC))

    pre_jobs = [(l, m, hh) for hh in range(2) for l in range(2) for m in range(NM)]
    # emit t-half 0 up front (needed from step 0); t-half 1 interleaved below
    for (l, m, hh) in [j for j in pre_jobs if j[2] == 0]:
        emit_pre_job(l, m, hh)
    pre_rest = [j for j in pre_jobs if j[2] == 1]

    # ---- main pools ----
    mpool = tc.tile_pool(name="main", bufs=3).__enter__()
    psA = tc.tile_pool(name="ps_gates", bufs=2, space="PSUM").__enter__()
    psL = tc.tile_pool(name="ps_logit", bufs=2, space="PSUM").__enter__()
    psS = tc.tile_pool(name="ps_small", bufs=2, space="PSUM").__enter__()
    ctx_pools += [mpool, psA, psL, psS]

    # ---- logits machinery ----
    logit_state = {}   # q -> dict(logits tile, sums tile, nlz tile)

    def emit_exp_job(q, w):
        st = logit_state.get(q)
        if st is None:
            st = {
                "logits": mpool.tile([128, V], BF16, tag="logits_q"),
                "sums": mpool.tile([128, NW], F32, tag="sums_q"),
                "nlz": mpool.tile([128, 1], F32, tag="nlz_q"),
            }
            logit_state[q] = st
        lp = psL.tile([128, VW], F32, tag="lps")
        for k in range(NK):
            nc.tensor.matmul(
                lp[:],
                HT[:, k * 512 + q * 128: k * 512 + (q + 1) * 128],
                fc_sb[:, k * V + w * VW: k * V + (w + 1) * VW],
                start=(k == 0), stop=(k == NK - 1) and not has_fcb)
        if has_fcb:
            nc.tensor.matmul(lp[:], ones_row16[:],
                             fcb_sb[:, w * VW:(w + 1) * VW],
                             start=False, stop=True)
        nc.vector.tensor_copy(st["logits"][:, w * VW:(w + 1) * VW], lp[:])
        ex = mpool.tile([128, VW], F32, tag="exp_scratch")
        nc.scalar.activation(ex[:], lp[:], AF.Exp,
                             accum_out=st["sums"][:, w:w + 1])

    def emit_stats_job(q):
        st = logit_state[q]
        z = mpool.tile([128, 1], F32, tag="z_q")
        nc.vector.tensor_reduce(z[:], st["sums"][:], mybir.AxisListType.X, OP.add)
        lz = mpool.tile([128, 1], F32, tag="lz_q")
        nc.scalar.activation(lz[:], z[:], AF.Ln)
        nc.vector.tensor_scalar_mul(st["nlz"][:], lz[:], -1.0)

    def emit_out_job(q, w):
        st = logit_state[q]
        ob = mpool.tile([128, VW], F32, tag="out_sb")
        nc.vector.tensor_scalar_add(ob[:], st["logits"][:, w * VW:(w + 1) * VW],
                                    st["nlz"][:, 0:1])
        # out rows bt = (t-1)*16 + b ; chunk q covers t-1 in [8q, 8q+8)
        dst = d["out"].ap().rearrange("b (q s) v -> q s b v", q=NBT)
        nc.sync.dma_start(dst[q, :, :, w * VW:(w + 1) * VW],
                          ob[:].rearrange("(s b) v -> s b v", b=BC))

    jobs = []  # flat job list in required order per q
    for q in range(NBT):
        jobs += [("exp", q, w) for w in range(NW)]
        jobs += [("stats", q, None)]
        jobs += [("out", q, w) for w in range(NW)]
    job_ready_t = {q: 8 * (q + 1) for q in range(NBT)}
    job_idx = [0]

    def run_job(j):
        kind, q, w = j
        if kind == "exp":
            emit_exp_job(q, w)
        elif kind == "stats":
            emit_stats_job(q)
        else:
            emit_out_job(q, w)

    def emit_ready_jobs(t, max_pe_jobs):
        pe_emitted = 0
        while job_idx[0] < len(jobs):
            kind, q, w = jobs[job_idx[0]]
            if t < job_ready_t[q]:
                break
            if kind == "exp":
                if pe_emitted >= max_pe_jobs:
                    break
                pe_emitted += 1
            run_job(jobs[job_idx[0]])
            job_idx[0] += 1

    # ---- recurrence ----
    for t in range(T):
        pre_sb = mpool.tile([128, 2 * NM * BC], F32, tag="pre")  # (l, m, b)
        src = preT_d[:].rearrange("l (b s) m p -> p l m b s", b=BC)
        nc.sync.dma_start(pre_sb[:], src[:, :, :, :, t:t + 1])

        h2 = None
        for l in range(2):
            rhs_t = hT if l == 0 else h1T
            gps = psA.tile([128, NM * BC], F32, tag="g")
            for m in range(NM):
                for k in range(NK):
                    nc.tensor.matmul(
                        gps[:, m * BC:(m + 1) * BC],
                        whh_sb[:, ((l * NK + k) * G + m * 128):((l * NK + k) * G + m * 128 + 128)],
                        rhs_t[:, k * BC:(k + 1) * BC],
                        start=(k == 0), stop=(k == NK - 1))
            gs = mpool.tile([128, NM * BC], F32, tag="gs")
            nc.vector.tensor_tensor(gs[:], gps[:],
                                    pre_sb[:, l * NM * BC:(l + 1) * NM * BC], OP.add)
            # gate blocks (m-major): i 0:64, f 64:128, o 128:192, g 192:256
            sg = mpool.tile([128, 192], F32, tag="sg")
            nc.scalar.activation(sg[:], gs[:, 0:192], AF.Sigmoid)
            tg = mpool.tile([128, 64], F32, tag="tg")
            nc.scalar.activation(tg[:], gs[:, 192:256], AF.Tanh)
            t1 = mpool.tile([128, 64], F32, tag="t1")
            nc.vector.tensor_tensor(t1[:], sg[:, 0:64], tg[:], OP.mult)
            t2 = mpool.tile([128, 64], F32, tag="t2")
            nc.vector.tensor_tensor(t2[:], sg[:, 64:128], cT[:], OP.mult)
            nc.vector.tensor_tensor(cT[:], t1[:], t2[:], OP.add)
            tch = mpool.tile([128, 64], F32, tag="tch")
            nc.scalar.activation(tch[:], cT[:], AF.Tanh)
            if l == 0:
                nc.vector.tensor_tensor(h1T[:], sg[:, 128:192], tch[:], OP.mult)
            else:
                h2 = mpool.tile([128, 64], F32, tag="h2")
                nc.vector.tensor_tensor(h2[:], sg[:, 128:192], tch[:], OP.mult)

        # ---- layernorm over d (partitions x 4 chunks) ----
        sq = mpool.tile([128, 64], F32, tag="sq")
        nc.vector.tensor_tensor(sq[:], h2[:], h2[:], OP.mult)
        sps = psS.tile([1, 128], F32, tag="s")
        nc.tensor.matmul(sps[0:1, 0:64], ones_col[:], h2[:], start=True, stop=True)
        nc.tensor.matmul(sps[0:1, 64:128], ones_col[:], sq[:], start=True, stop=True)
        ssb = mpool.tile([1, 128], F32, tag="ssb")
        nc.vector.tensor_copy(ssb[:], sps[:])
        stat = mpool.tile([1, 64], F32, tag="stat")  # mu|rs|ms/var|mu2/sd
        nc.vector.tensor_reduce(stat[0:1, 0:16],
                                ssb[0:1, 0:64].rearrange("p (c b) -> p b c", c=NK),
                                mybir.AxisListType.X, OP.add)
        nc.vector.tensor_reduce(stat[0:1, 32:48],
                                ssb[0:1, 64:128].rearrange("p (c b) -> p b c", c=NK),
                                mybir.AxisListType.X, OP.add)
        nc.vector.tensor_scalar_mul(stat[0:1, 0:16], stat[0:1, 0:16], 1.0 / D)
        nc.vector.tensor_scalar_mul(stat[0:1, 32:48], stat[0:1, 32:48], 1.0 / D)
        nc.vector.tensor_tensor(stat[0:1, 48:64], stat[0:1, 0:16],
                                stat[0:1, 0:16], OP.mult)
        nc.vector.tensor_tensor(stat[0:1, 32:48], stat[0:1, 32:48],
                                stat[0:1, 48:64], OP.subtract)
        nc.scalar.activation(stat[0:1, 48:64], stat[0:1, 32:48], AF.Sqrt,
                             bias=LN_EPS)
        nc.vector.reciprocal(stat[0:1, 16:32], stat[0:1, 48:64])
        bps = psS.tile([128, 32], F32, tag="b")
        nc.tensor.matmul(bps[:], ones_row[:], stat[0:1, 0:32], start=True, stop=True)
        mu_bc = _ap_insert_bcast(bps[:, 0:16], 0, NK)    # [128, 4c(bcast), 16b]
        rs_bc = _ap_insert_bcast(bps[:, 16:32], 0, NK)
        d0 = mpool.tile([128, 64], F32, tag="d0")
        nc.vector.tensor_tensor(
            d0[:].rearrange("p (c b) -> p c b", c=NK), h2[:].rearrange("p (c b) -> p c b", c=NK), mu_bc, OP.subtract)
        if has_ln:
            hl = mpool.tile([128, 64], F32, tag="hl")
            nc.vector.tensor_tensor(
                hl[:].rearrange("p (c b) -> p c b", c=NK),
                d0[:].rearrange("p (c b) -> p c b", c=NK), rs_bc, OP.mult)
            gg = _ap_insert_bcast(lng_sb[:], 1, BC)      # [128, 4c, 16b(bcast)]
            bb = _ap_insert_bcast(lnb_sb[:], 1, BC)
            hl2 = mpool.tile([128, 64], F32, tag="hl2")
            nc.vector.tensor_tensor(
                hl2[:].rearrange("p (c b) -> p c b", c=NK),
                hl[:].rearrange("p (c b) -> p c b", c=NK), gg, OP.mult)
            nc.vector.tensor_tensor(
                hT[:].rearrange("p (c b) -> p c b", c=NK),
                hl2[:].rearrange("p (c b) -> p c b", c=NK), bb, OP.add)
        else:
            nc.vector.tensor_tensor(
                hT[:].rearrange("p (c b) -> p c b", c=NK),
                d0[:].rearrange("p (c b) -> p c b", c=NK), rs_bc, OP.mult)
        if t > 0:
            nc.vector.tensor_copy(
                HT[:].rearrange("p (k n) -> p k n", k=NK)[:, :, (t - 1) * BC:t * BC],
                hT[:].rearrange("p (k b) -> p k b", k=NK))

        # interleave deferred parallel work
        if pre_rest and t < 28:
            emit_pre_job(*pre_rest.pop(0))
            if pre_rest and t < 14:
                emit_pre_job(*pre_rest.pop(0))
        emit_ready_jobs(t, max_pe_jobs=2)

    while pre_rest:
        emit_pre_job(*pre_rest.pop(0))
    emit_ready_jobs(T, max_pe_jobs=10 ** 9)
    assert job_idx[0] == len(jobs)

    for p in reversed(ctx_pools):
        p.__exit__(None, None, None)


def _build(flags):
    nc = bacc.Bacc("TRN2", target_bir_lowering=False, debug=False, num_devices=1)
    d = {}
    d["cap"] = nc.dram_tensor("cap", [BC * LCAP], I32, kind="ExternalInput")
    d["featT"] = nc.dram_tensor("featT", [D, BC], BF16, kind="ExternalInput")
    d["embW"] = nc.dram_tensor("embW", [V, D], BF16, kind="ExternalInput")
    wih = nc.dram_tensor("wihT", [2, D, G], BF16, kind="ExternalInput")
    whh = nc.dram_tensor("whhT", [2, D, G], BF16, kind="ExternalInput")
    d["wihT"] = [wih.ap()[l] for l in range(2)]
    d["whhT"] = [whh.ap()[l] for l in range(2)]
    d["biasT"] = nc.dram_tensor("biasT", [128, 32], F32, kind="ExternalInput")
    d["fcWT"] = nc.dram_tensor("fcWT", [D, V], BF16, kind="ExternalInput")
    if flags["has_fcb"]:
        d["fcb"] = nc.dram_tensor("fcb", [V], BF16, kind="ExternalInput")
    if flags["has_ln"]:
        d["lng"] = nc.dram_tensor("lng", [D], F32, kind="ExternalInput")
        d["lnb"] = nc.dram_tensor("lnb", [D], F32, kind="ExternalInput")
    d["out"] = nc.dram_tensor("out", [BC, LCAP, V], F32, kind="ExternalOutput")

    with tile.TileContext(nc) as tc:
        _emit(nc, tc, d, flags)
    nc.compile()
    return nc


def _prep_inputs(features, caption, emb_W, W_ih, W_hh, b_ih, b_hh,
                 ln_g, ln_b, fc_W, fc_b):
    """Host-side marshaling: shard, permute gate order, transpose, cast."""
    features = np.asarray(features, np.float32)
    caption = np.asarray(caption)
    emb_W = np.asarray(emb_W, np.float32)
    W_ih = np.asarray(W_ih, np.float32)
    W_hh = np.asarray(W_hh, np.float32)
    b_sum = np.asarray(b_ih, np.float32) + np.asarray(b_hh, np.float32)
    ln_g = np.asarray(ln_g, np.float32)
    ln_b = np.asarray(ln_b, np.float32)
    fc_W = np.asarray(fc_W, np.float32)
    fc_b = np.asarray(fc_b, np.float32)

    # gate order [i,f,g,o] -> [i,f,o,g]
    perm = np.concatenate([np.arange(0, 2 * D), np.arange(3 * D, 4 * D),
                           np.arange(2 * D, 3 * D)])
    wihT = np.ascontiguousarray(
        W_ih[:, perm, :].transpose(0, 2, 1)).astype(bf16)      # [2, D, G]
    whhT = np.ascontiguousarray(
        W_hh[:, perm, :].transpose(0, 2, 1)).astype(bf16)
    biasT = np.ascontiguousarray(
        b_sum[:, perm].reshape(2, NM, 128).transpose(2, 0, 1).reshape(128, 32))
    fcWT = np.ascontiguousarray(fc_W.T).astype(bf16)           # [D, V]
    embW = emb_W.astype(bf16)

    flags = {
        "has_fcb": bool(np.any(fc_b)),
        "has_ln": not (np.all(ln_g == 1.0) and np.all(ln_b == 0.0)),
    }

    in_maps = []
    for c in range(NC):
        bs = slice(c * BC, (c + 1) * BC)
        m = {
            "cap": np.ascontiguousarray(caption[bs]).reshape(-1).astype(np.int32),
            "featT": np.ascontiguousarray(features[bs, 0, :].T).astype(bf16),
            "embW": embW,
            "wihT": wihT,
            "whhT": whhT,
            "biasT": biasT,
            "fcWT": fcWT,
        }
        if flags["has_fcb"]:
            m["fcb"] = fc_b.astype(bf16)
        if flags["has_ln"]:
            m["lng"] = ln_g
            m["lnb"] = ln_b
        in_maps.append(m)
    return in_maps, flags


_CACHE = {}


def _get_compiled(flags):
    key = tuple(sorted(flags.items()))
    if key not in _CACHE:
        _CACHE[key] = {"nc": _build(flags)}
    return _CACHE[key]


def _get_runner(entry):
    """Cached jitted 8-core SPMD runner (mirrors bass2jax.run_bass_via_pjrt,
    but reusable across calls so the NEFF compiles once)."""
    if "runner" in entry:
        return entry["runner"]
    import jax
    from jax.sharding import Mesh, PartitionSpec
    from jax.experimental.shard_map import shard_map
    from concourse import bass2jax
    from concourse.bass2jax import _bass_exec_p, install_neuronx_cc_hook

    nc = entry["nc"]
    install_neuronx_cc_hook()
    partition_name = (nc.partition_id_tensor.name
                      if nc.partition_id_tensor else None)
    in_names, out_names, out_avals, zero_outs = [], [], [], []
    for alloc in nc.m.functions[0].allocations:
        if not isinstance(alloc, mybir.MemoryLocationSet):
            continue
        name = alloc.memorylocations[0].name
        if alloc.kind == "ExternalInput":
            if name != partition_name:
                in_names.append(name)
        elif alloc.kind == "ExternalOutput":
            shape = tuple(alloc.tensor_shape)
            dtype = mybir.dt.np(alloc.dtype)
            out_names.append(name)
            out_avals.append(jax.core.ShapedArray(shape, dtype))
            zero_outs.append(np.zeros(shape, dtype))
    n_params = len(in_names)
    all_in_names = in_names + out_names
    if partition_name is not None:
        all_in_names = all_in_names + [partition_name]

    def _body(*args):
        operands = list(args)
        if partition_name is not None:
            operands.append(bass2jax.partition_id_tensor())
        outs = _bass_exec_p.bind(
            *operands,
            out_avals=tuple(out_avals),
            in_names=tuple(all_in_names),
            out_names=tuple(out_names),
            lowering_input_output_aliases=(),
            sim_require_finite=True,
            sim_require_nnan=True,
            nc=nc,
        )
        return tuple(outs)

    devices = jax.devices()[:NC]
    mesh = Mesh(np.asarray(devices), ("core",))
    n_outs = len(out_names)
    sharded = jax.jit(
        shard_map(_body, mesh=mesh,
                  in_specs=(PartitionSpec("core"),) * (n_params + n_outs),
                  out_specs=(PartitionSpec("core"),) * n_outs,
                  check_rep=False),
        keep_unused=True)

    def run(in_maps):
        concat_in = [np.concatenate([np.asarray(m[n]) for m in in_maps], axis=0)
                     for n in in_names]
        concat_zero = [np.zeros((NC * z.shape[0], *z.shape[1:]), z.dtype)
                       for z in zero_outs]
        out_arrs = sharded(*concat_in, *concat_zero)
        return [
            {n: np.asarray(out_arrs[i]).reshape(NC, *out_avals[i].shape)[c]
             for i, n in enumerate(out_names)}
            for c in range(NC)
        ]

    entry["runner"] = run
    return run


def kernel(**inputs):
    in_maps, flags = _prep_inputs(**inputs)
    entry = _get_compiled(flags)
    results = _get_runner(entry)(in_maps)
    return np.concatenate([r["out"] for r in results], axis=0)


# revision 24
# speedup vs baseline: 2892.6353x; 2892.6353x over previous
"""Trainium2 Bass kernel for nn_DecoderRNN (LSTM decoder + vocab projection).

Sharding: batch 128 -> 16 per core across 8 cores (data parallel). LSTM
weights/embedding/fc replicated. Matmuls in fp16 with split-precision
corrections (fp32 PSUM accumulate):
  - W_hh = Whi + Wlo (both fp16), h = h_hi + h_lo (both fp16):
      gates ~= Whi@[h_hi|h_lo] + Wlo@h_hi   (drops only Wlo@h_lo ~ 2^-22)
  - W_ih split the same way (one-time precompute cost).
  - fc_W single fp16 (output-stage error does not compound).

Per-core layout: recurrent state kept transposed (D on partitions, batch on
free dim) so the h @ Whh^T matmuls chain without per-step transposes:
  gates^T[m-chunk 128, b] = WhhT_tile[k 128, m 128]^T @ hT[k 128, b]
Gate order host-permuted to [i, f, o, g] so sigmoid runs on one contiguous
block. Input-side gate preactivations (x_t @ Wih^T + biases) are precomputed
for all timesteps up front and spilled to DRAM, streamed back per step.
Logits per 128-token (t,b) chunk with H^T tiles stationary and fc_W^T
streaming; log-softmax without max-subtraction (h is layernormed so logits
are small), exp+sum via ACT accum_out.
"""

import numpy as np
import ml_dtypes  # noqa: F401

import concourse.bass as bass
import concourse.mybir as mybir
import concourse.tile as tile
from concourse import bacc
from concourse.masks import make_identity

F32 = mybir.dt.float32
F16 = mybir.dt.float16
I32 = mybir.dt.int32
AF = mybir.ActivationFunctionType
OP = mybir.AluOpType

NC = 8          # cores
B = 128         # global batch
BC = B // NC    # batch per core (16)
T = 33          # timesteps (1 feature + 32 caption)
LCAP = 32
D = 512
V = 10000
G = 4 * D       # 2048 gate dim
NK = D // 128   # 4 k-chunks
NM = G // 128   # 16 m-chunks
TOK = BC * T    # 528 tokens per core
NBT = BC * LCAP // 128  # 4 output (t,b) chunks of 128
NW = 20         # vocab chunks of 500
VW = V // NW    # 500
LN_EPS = 1e-5

f16 = np.float16


def _ap_insert_bcast(ap_obj, pos, count):
    """Insert a stride-0 dim of size `count` at free-dim position `pos`."""
    dims = [list(dd) for dd in ap_obj.ap]
    dims.insert(1 + pos, [0, count])
    return bass.AP(ap_obj.tensor, ap_obj.offset, dims)


def _emit(nc, tc, d, flags):
    """Emit the full per-core program. d: dict of dram tensors."""
    from contextlib import ExitStack

    has_fcb = flags["has_fcb"]
    has_ln = flags["has_ln"]

    es = ExitStack()
    cpool = es.enter_context(tc.tile_pool(name="const", bufs=1))
    ppool = es.enter_context(tc.tile_pool(name="persist", bufs=1))
    dpool = es.enter_context(tc.tile_pool(name="dram", bufs=1, space="DRAM"))

    ident = cpool.tile([128, 128], F16)
    make_identity(nc, ident[:])
    ones_col = cpool.tile([128, 1], F32)   # lhsT for partition sums
    nc.vector.memset(ones_col[:], 1.0)
    ones_row = cpool.tile([1, 128], F32)   # lhsT for broadcast matmul
    nc.vector.memset(ones_row[:], 1.0)
    eps_sb = cpool.tile([1, 1], F32)
    nc.vector.memset(eps_sb[:], LN_EPS)

    # ---- persistent tiles ----
    # whh_sb free layout: (l, hl, k, g)
    whh_sb = ppool.tile([128, 2 * 2 * NK * G], F16)   # 64KB/part
    for l in range(2):
        for hl in range(2):
            nc.sync.dma_start(
                whh_sb[:, (l * 2 + hl) * NK * G:(l * 2 + hl + 1) * NK * G]
                .rearrange("p (k g) -> p k g", k=NK),
                d["whhT"].ap()[l, hl].rearrange("(k p) g -> p k g", p=128))

    def whh_tile(l, hl, k, m):
        off = ((l * 2 + hl) * NK + k) * G + m * 128
        return whh_sb[:, off:off + 128]

    bias_sb = ppool.tile([128, 32], F32)           # (l, m)
    nc.sync.dma_start(bias_sb[:], d["biasT"].ap())
    fc_sb = ppool.tile([128, NK * V], F16)         # (k, v) 80KB/part
    nc.sync.dma_start(fc_sb[:].rearrange("p (k v) -> p k v", k=NK),
                      d["fcWT"].ap().rearrange("(k p) v -> p k v", p=128))
    if has_fcb:
        fcb_sb = ppool.tile([1, V], F16)
        nc.sync.dma_start(fcb_sb[:], d["fcb"].ap())
        ones_row16 = cpool.tile([1, 128], F16)
        nc.vector.memset(ones_row16[:], 1.0)
    if has_ln:
        lng_sb = ppool.tile([128, NK], F32)
        nc.sync.dma_start(lng_sb[:], d["lng"].ap())
        lnb_sb = ppool.tile([128, NK], F32)
        nc.sync.dma_start(lnb_sb[:], d["lnb"].ap())

    HT = ppool.tile([128, NK * BC * LCAP], F16)    # (k, bt) bt=(t-1)*16+b
    # state: (k, [hi(16) | lo(16)]) per k-chunk
    hT = ppool.tile([128, NK * 2 * BC], F16)
    h1T = ppool.tile([128, NK * 2 * BC], F16)
    cT = ppool.tile([128, NK * BC], F32)
    nc.vector.memset(hT[:], 0.0)
    nc.vector.memset(cT[:], 0.0)

    preT_d = dpool.tile([T, 128, 2 * NM * BC], F32)  # spill (t, p, (l m b))

    # ---- prologue: embedding gather + transpose into xT ----
    prolog_es = ExitStack()
    lpool = prolog_es.enter_context(tc.tile_pool(name="prolog", bufs=2))

    xT = lpool.tile([128, NK * TOK], F16, bufs=1)  # (k, tok) tok=b*33+t
    xT4 = xT[:].rearrange("p (k b s) -> p k b s", k=NK, b=BC)  # s = t 0..32

    cap_sb = lpool.tile([128, 4], I32, tag="cap")
    nc.sync.dma_start(cap_sb[:], d["cap"].ap())

    ftmp = lpool.tile([128, NK * BC], F16, tag="ftmp")
    nc.sync.dma_start(ftmp[:], d["featT"].ap())
    nc.vector.tensor_copy(xT4[:, :, :, 0],
                          ftmp[:].rearrange("p (k b) -> p k b", k=NK))

    with tc.tile_pool(name="ps_t", bufs=2, space="PSUM") as ps_t:
        for g in range(4):
            gt = lpool.tile([128, D], F16, tag="gather")
            nc.gpsimd.indirect_dma_start(
                out=gt[:], out_offset=None,
                in_=d["embW"].ap(),
                in_offset=bass.IndirectOffsetOnAxis(ap=cap_sb[:, g:g + 1], axis=0))
            for k in range(NK):
                tps = ps_t.tile([128, 128], F16, tag="tps")
                nc.tensor.transpose(tps[:], gt[:, k * 128:(k + 1) * 128], ident[:])
                # tokens e = g*128 + r*32 + s -> b = 4g+r, slot = b*33 + 1 + s
                nc.vector.tensor_copy(
                    xT4[:, k, 4 * g:4 * g + 4, 1:33],
                    tps[:].rearrange("p (r s) -> p r s", r=4))

    # ---- pre-gate precompute (all up front): jobs (l, m, t-half) ----
    t_halves = [(0, 17), (17, 16)]  # (t0, nt)
    with tc.tile_pool(name="ps_pre", bufs=2, space="PSUM") as lps:
        for l in range(2):
            wih_hi = lpool.tile([128, NK * G], F16, tag="wih_hi", name="wih_hi", bufs=1)
            wih_lo = lpool.tile([128, NK * G], F16, tag="wih_lo", name="wih_lo", bufs=1)
            for hl, wt in ((0, wih_hi), (1, wih_lo)):
                nc.sync.dma_start(
                    wt[:].rearrange("p (k g) -> p k g", k=NK),
                    d["wihT"].ap()[l, hl].rearrange("(k p) g -> p k g", p=128))
            for m in range(NM):
                for hh in range(2):
                    t0, nt = t_halves[hh]
                    n = BC * nt
                    pps = lps.tile([128, 272], F32, tag="pps")
                    for hl, wt in ((0, wih_hi), (1, wih_lo)):
                        for k in range(NK):
                            rhs = xT4[:, k, :, t0:t0 + nt].rearrange(
                                "p b s -> p s b")
                            nc.tensor.matmul(
                                pps[:, :n],
                                wt[:, (k * G + m * 128):(k * G + m * 128 + 128)],
                                rhs, start=(hl == 0 and k == 0),
                                stop=(hl == 1 and k == NK - 1))
                    sc = lpool.tile([128, 272], F32, tag="psc")
                    nc.scalar.activation(
                        sc[:, :n], pps[:, :n], AF.Identity,
                        bias=bias_sb[:, l * NM + m:l * NM + m + 1])
                    # DRAM (t, p, (l m b)); sc iter (t, b)
                    dst = preT_d[:].rearrange(
                        "s p (l m b) -> p l m s b", l=2, m=NM)[:, l, m,
                                                              t0:t0 + nt, :]
                    nc.sync.dma_start(
                        dst, sc[:, :n].rearrange("p (s b) -> p s b", b=BC))

    prolog_es.close()

    # ---- main pools ----
    mpool = es.enter_context(tc.tile_pool(name="main", bufs=3))
    psA = es.enter_context(tc.tile_pool(name="ps_gates", bufs=2, space="PSUM"))
    psL = es.enter_context(tc.tile_pool(name="ps_logit", bufs=2, space="PSUM"))
    psS = es.enter_context(tc.tile_pool(name="ps_small", bufs=1, space="PSUM"))

    # ---- logits machinery ----
    logit_state = {}   # q -> dict(logits tile, sums tile, nlz tile)

    def emit_exp_job(q, w):
        st = logit_state.get(q)
        if st is None:
            st = {
                "logits": mpool.tile([128, V], F16, tag="logits_q",
                                     name="logits_q", bufs=1),
                "sums": mpool.tile([128, NW], F32, tag="sums_q", name="sums_q"),
                "nlz": mpool.tile([128, 1], F32, tag="nlz_q", name="nlz_q"),
            }
            logit_state[q] = st
        lp = psL.tile([128, VW], F32, tag="lps")
        for k in range(NK):
            nc.tensor.matmul(
                lp[:],
                HT[:, k * 512 + q * 128: k * 512 + (q + 1) * 128],
                fc_sb[:, k * V + w * VW: k * V + (w + 1) * VW],
                start=(k == 0), stop=(k == NK - 1) and not has_fcb)
        if has_fcb:
            nc.tensor.matmul(lp[:], ones_row16[:],
                             fcb_sb[:, w * VW:(w + 1) * VW],
                             start=False, stop=True)
        nc.vector.tensor_copy(st["logits"][:, w * VW:(w + 1) * VW], lp[:])
        ex = mpool.tile([128, VW], F32, tag="exp_scratch", bufs=2)
        nc.scalar.activation(ex[:], lp[:], AF.Exp,
                             accum_out=st["sums"][:, w:w + 1])

    def emit_stats_job(q):
        st = logit_state[q]
        z = mpool.tile([128, 1], F32, tag="z_q")
        nc.vector.tensor_reduce(z[:], st["sums"][:], mybir.AxisListType.X, OP.add)
        lz = mpool.tile([128, 1], F32, tag="lz_q")
        nc.scalar.activation(lz[:], z[:], AF.Ln)
        nc.vector.tensor_scalar_mul(st["nlz"][:], lz[:], -1.0)

    def emit_out_job(q, w):
        st = logit_state[q]
        ob = mpool.tile([128, VW], F32, tag="out_sb")
        nc.vector.tensor_scalar_add(ob[:], st["logits"][:, w * VW:(w + 1) * VW],
                                    st["nlz"][:, 0:1])
        # out rows bt = (t-1)*16 + b ; chunk q covers t-1 in [8q, 8q+8)
        dst = d["out"].ap().rearrange("b (q s) v -> q s b v", q=NBT)
        nc.sync.dma_start(dst[q, :, :, w * VW:(w + 1) * VW], ob[:])

    jobs = []  # flat job list in required order per q
    for q in range(NBT):
        jobs += [("exp", q, w) for w in range(NW)]
        jobs += [("stats", q, None)]
        jobs += [("out", q, w) for w in range(NW)]
    job_ready_t = {q: 8 * (q + 1) for q in range(NBT)}
    job_idx = [0]

    def run_job(j):
        kind, q, w = j
        if kind == "exp":
            emit_exp_job(q, w)
        elif kind == "stats":
            emit_stats_job(q)
        else:
            emit_out_job(q, w)

    def emit_ready_jobs(t, max_pe_jobs):
        pe_emitted = 0
        while job_idx[0] < len(jobs):
            kind, q, w = jobs[job_idx[0]]
            if t < job_ready_t[q]:
                break
            if kind == "exp":
                if pe_emitted >= max_pe_jobs:
                    break
                pe_emitted += 1
            run_job(jobs[job_idx[0]])
            job_idx[0] += 1

    def write_split(dst_split, src_f32, tag):
        """dst_split layout [128, (NK, 2, BC)]: hi|lo fp16 halves of src."""
        d3 = dst_split[:].rearrange("p (k hl b) -> p k hl b", k=NK, hl=2)
        s3 = src_f32[:].rearrange("p (k b) -> p k b", k=NK)
        nc.vector.tensor_copy(d3[:, :, 0, :], s3)           # hi = fp16(x)
        back = mpool.tile([128, NK * BC], F32, tag=tag, name=tag)
        nc.vector.tensor_copy(back[:], d3[:, :, 0, :])      # hi back to f32
        nc.vector.tensor_tensor(d3[:, :, 1, :], s3,
                                back[:].rearrange("p (k b) -> p k b", k=NK),
                                OP.subtract)                 # lo = x - hi

    # ---- recurrence ----
    for t in range(T):
        pre_sb = mpool.tile([128, 2 * NM * BC], F32, tag="pre")  # (l, m, b)
        nc.sync.dma_start(pre_sb[:], preT_d[t])

        h2 = None
        for l in range(2):
            rhs_t = hT if l == 0 else h1T
            # psum layout per m: [hi-products(16) | lo-products(16)]
            gps = psA.tile([128, NM * 2 * BC], F32, tag="g")
            for m in range(NM):
                reg32 = gps[:, m * 32:(m + 1) * 32]
                reg16 = gps[:, m * 32:m * 32 + 16]
                for k in range(NK):
                    # Whi @ [h_hi | h_lo]  (N=32)
                    nc.tensor.matmul(
                        reg32, whh_tile(l, 0, k, m),
                        rhs_t[:, k * 32:(k + 1) * 32],
                        start=(k == 0), stop=False, skip_group_check=True)
                for k in range(NK):
                    # Wlo @ h_hi  (N=16) accumulated onto hi half
                    nc.tensor.matmul(
                        reg16, whh_tile(l, 1, k, m),
                        rhs_t[:, k * 32:k * 32 + 16],
                        start=False, stop=(k == NK - 1), skip_group_check=True)
            # gates = (hi-half + pre) + lo-half (one PSUM operand per op)
            g3 = gps[:].rearrange("p (m hl b) -> p hl m b", m=NM, hl=2)
            gs = mpool.tile([128, NM * BC], F32, tag="gs")
            gs3 = gs[:].rearrange("p (m b) -> p m b", m=NM)
            pre3 = pre_sb[:, l * NM * BC:(l + 1) * NM * BC].rearrange(
                "p (m b) -> p m b", m=NM)
            nc.vector.tensor_tensor(gs3, g3[:, 0], pre3, OP.add)
            nc.vector.tensor_tensor(gs3, gs3, g3[:, 1], OP.add)
            # gate blocks (m-major): i 0:64, f 64:128, o 128:192, g 192:256
            sg = mpool.tile([128, 192], F32, tag="sg")
            nc.scalar.activation(sg[:], gs[:, 0:192], AF.Sigmoid)
            tg = mpool.tile([128, 64], F32, tag="tg")
            nc.scalar.activation(tg[:], gs[:, 192:256], AF.Tanh)
            t1 = mpool.tile([128, 64], F32, tag="t1")
            nc.vector.tensor_tensor(t1[:], sg[:, 0:64], tg[:], OP.mult)
            t2 = mpool.tile([128, 64], F32, tag="t2")
            nc.vector.tensor_tensor(t2[:], sg[:, 64:128], cT[:], OP.mult)
            nc.vector.tensor_tensor(cT[:], t1[:], t2[:], OP.add)
            tch = mpool.tile([128, 64], F32, tag="tch")
            nc.scalar.activation(tch[:], cT[:], AF.Tanh)
            hval = mpool.tile([128, 64], F32, tag="hval", name="hval")
            nc.vector.tensor_tensor(hval[:], sg[:, 128:192], tch[:], OP.mult)
            if l == 0:
                write_split(h1T, hval, "back1")
            else:
                h2 = hval

        # ---- layernorm over d (partitions x 4 chunks) ----
        sq = mpool.tile([128, 64], F32, tag="sq")
        nc.vector.tensor_tensor(sq[:], h2[:], h2[:], OP.mult)
        sps = psS.tile([1, 128], F32, tag="s")
        nc.tensor.matmul(sps[0:1, 0:64], ones_col[:], h2[:], start=True, stop=True)
        nc.tensor.matmul(sps[0:1, 64:128], ones_col[:], sq[:], start=True, stop=True)
        ssb = mpool.tile([1, 128], F32, tag="ssb")
        nc.vector.tensor_copy(ssb[:], sps[:])
        stat = mpool.tile([1, 64], F32, tag="stat")  # mu|rs|ms/var|mu2/sd
        nc.vector.tensor_reduce(stat[0:1, 0:16],
                                ssb[0:1, 0:64].rearrange("p (c b) -> p b c", c=NK),
                                mybir.AxisListType.X, OP.add)
        nc.vector.tensor_reduce(stat[0:1, 32:48],
                                ssb[0:1, 64:128].rearrange("p (c b) -> p b c", c=NK),
                                mybir.AxisListType.X, OP.add)
        nc.vector.tensor_scalar_mul(stat[0:1, 0:16], stat[0:1, 0:16], 1.0 / D)
        nc.vector.tensor_scalar_mul(stat[0:1, 32:48], stat[0:1, 32:48], 1.0 / D)
        nc.vector.tensor_tensor(stat[0:1, 48:64], stat[0:1, 0:16],
                                stat[0:1, 0:16], OP.mult)
        nc.vector.tensor_tensor(stat[0:1, 32:48], stat[0:1, 32:48],
                                stat[0:1, 48:64], OP.subtract)
        nc.scalar.activation(stat[0:1, 48:64], stat[0:1, 32:48], AF.Sqrt,
                             bias=eps_sb[0:1, 0:1])
        nc.vector.reciprocal(stat[0:1, 16:32], stat[0:1, 48:64])
        bps = psS.tile([128, 32], F32, tag="b")
        nc.tensor.matmul(bps[:], ones_row[:], stat[0:1, 0:32], start=True, stop=True)
        mu_bc = _ap_insert_bcast(bps[:, 0:16], 0, NK)    # [128, 4c(bcast), 16b]
        rs_bc = _ap_insert_bcast(bps[:, 16:32], 0, NK)
        d0 = mpool.tile([128, 64], F32, tag="d0")
        nc.vector.tensor_tensor(
            d0[:].rearrange("p (c b) -> p c b", c=NK),
            h2[:].rearrange("p (c b) -> p c b", c=NK), mu_bc, OP.subtract)
        hln = mpool.tile([128, 64], F32, tag="hln", name="hln")
        nc.vector.tensor_tensor(
            hln[:].rearrange("p (c b) -> p c b", c=NK),
            d0[:].rearrange("p (c b) -> p c b", c=NK), rs_bc, OP.mult)
        if has_ln:
            gg = _ap_insert_bcast(lng_sb[:], 1, BC)      # [128, 4c, 16b(bcast)]
            bb = _ap_insert_bcast(lnb_sb[:], 1, BC)
            hl2 = mpool.tile([128, 64], F32, tag="hl2")
            nc.vector.tensor_tensor(
                hl2[:].rearrange("p (c b) -> p c b", c=NK),
                hln[:].rearrange("p (c b) -> p c b", c=NK), gg, OP.mult)
            nc.vector.tensor_tensor(
                hln[:].rearrange("p (c b) -> p c b", c=NK),
                hl2[:].rearrange("p (c b) -> p c b", c=NK), bb, OP.add)
        write_split(hT, hln, "back0")
        if t > 0:
            nc.vector.tensor_copy(
                HT[:].rearrange("p (k n) -> p k n", k=NK)[:, :, (t - 1) * BC:t * BC],
                hT[:].rearrange("p (k hl b) -> p k hl b", k=NK, hl=2)[:, :, 0, :])

        emit_ready_jobs(t, max_pe_jobs=2)

    emit_ready_jobs(T, max_pe_jobs=10 ** 9)
    assert job_idx[0] == len(jobs)

    es.close()


def _build(flags):
    nc = bacc.Bacc("TRN2", target_bir_lowering=False, debug=False, num_devices=1)
    d = {}
    d["cap"] = nc.dram_tensor("cap", [128, 4], I32, kind="ExternalInput")
    d["featT"] = nc.dram_tensor("featT", [128, NK * BC], F16, kind="ExternalInput")
    d["embW"] = nc.dram_tensor("embW", [V, D], F16, kind="ExternalInput")
    d["wihT"] = nc.dram_tensor("wihT", [2, 2, D, G], F16, kind="ExternalInput")
    d["whhT"] = nc.dram_tensor("whhT", [2, 2, D, G], F16, kind="ExternalInput")
    d["biasT"] = nc.dram_tensor("biasT", [128, 32], F32, kind="ExternalInput")
    d["fcWT"] = nc.dram_tensor("fcWT", [D, V], F16, kind="ExternalInput")
    if flags["has_fcb"]:
        d["fcb"] = nc.dram_tensor("fcb", [1, V], F16, kind="ExternalInput")
    if flags["has_ln"]:
        d["lng"] = nc.dram_tensor("lng", [128, NK], F32, kind="ExternalInput")
        d["lnb"] = nc.dram_tensor("lnb", [128, NK], F32, kind="ExternalInput")
    d["out"] = nc.dram_tensor("out", [BC, LCAP, V], F32, kind="ExternalOutput")

    with tile.TileContext(nc) as tc:
        _emit(nc, tc, d, flags)
    nc.compile()
    return nc


def _split16(x):
    """x (f32) -> stacked [2, ...] fp16 hi/lo pair."""
    hi = x.astype(f16)
    lo = (x - hi.astype(np.float32)).astype(f16)
    return np.stack([hi, lo])


def _prep_inputs(features, caption, emb_W, W_ih, W_hh, b_ih, b_hh,
                 ln_g, ln_b, fc_W, fc_b):
    """Host-side marshaling: shard, permute gate order, transpose, cast."""
    features = np.asarray(features, np.float32)
    caption = np.asarray(caption)
    emb_W = np.asarray(emb_W, np.float32)
    W_ih = np.asarray(W_ih, np.float32)
    W_hh = np.asarray(W_hh, np.float32)
    b_sum = np.asarray(b_ih, np.float32) + np.asarray(b_hh, np.float32)
    ln_g = np.asarray(ln_g, np.float32)
    ln_b = np.asarray(ln_b, np.float32)
    fc_W = np.asarray(fc_W, np.float32)
    fc_b = np.asarray(fc_b, np.float32)

    # gate order [i,f,g,o] -> [i,f,o,g]
    perm = np.concatenate([np.arange(0, 2 * D), np.arange(3 * D, 4 * D),
                           np.arange(2 * D, 3 * D)])
    wihT = _split16(np.ascontiguousarray(
        W_ih[:, perm, :].transpose(0, 2, 1)))                # [2hl, 2l, D, G]
    wihT = np.ascontiguousarray(wihT.transpose(1, 0, 2, 3))  # [2l, 2hl, D, G]
    whhT = _split16(np.ascontiguousarray(
        W_hh[:, perm, :].transpose(0, 2, 1)))
    whhT = np.ascontiguousarray(whhT.transpose(1, 0, 2, 3))
    biasT = np.ascontiguousarray(
        b_sum[:, perm].reshape(2, NM, 128).transpose(2, 0, 1).reshape(128, 32))
    fcWT = np.ascontiguousarray(fc_W.T).astype(f16)          # [D, V]
    embW = emb_W.astype(f16)

    flags = {
        "has_fcb": bool(np.any(fc_b)),
        "has_ln": not (np.all(ln_g == 1.0) and np.all(ln_b == 0.0)),
    }

    in_maps = []
    for c in range(NC):
        bs = slice(c * BC, (c + 1) * BC)
        m = {
            "cap": np.ascontiguousarray(caption[bs]).reshape(4, 128).T
            .copy().astype(np.int32),
            "featT": np.ascontiguousarray(
                features[bs, 0, :].T.reshape(NK, 128, BC).transpose(1, 0, 2)
                .reshape(128, NK * BC)).astype(f16),
            "embW": embW,
            "wihT": wihT,
            "whhT": whhT,
            "biasT": biasT,
            "fcWT": fcWT,
        }
        if flags["has_fcb"]:
            m["fcb"] = fc_b.reshape(1, V).astype(f16)
        if flags["has_ln"]:
            m["lng"] = np.ascontiguousarray(ln_g.reshape(NK, 128).T)
            m["lnb"] = np.ascontiguousarray(ln_b.reshape(NK, 128).T)
        in_maps.append(m)
    return in_maps, flags


_CACHE = {}


def _get_compiled(flags):
    key = tuple(sorted(flags.items()))
    if key not in _CACHE:
        _CACHE[key] = {"nc": _build(flags)}
    return _CACHE[key]


def _get_runner(entry):
    """Cached jitted 8-core SPMD runner (mirrors bass2jax.run_bass_via_pjrt,
    but reusable across calls so the NEFF compiles once)."""
    if "runner" in entry:
        return entry["runner"]
    import jax
    from jax.sharding import Mesh, PartitionSpec
    from jax.experimental.shard_map import shard_map
    from concourse import bass2jax
    from concourse.bass2jax import _bass_exec_p, install_neuronx_cc_hook

    nc = entry["nc"]
    install_neuronx_cc_hook()
    partition_name = (nc.partition_id_tensor.name
                      if nc.partition_id_tensor else None)
    in_names, out_names, out_avals, zero_outs = [], [], [], []
    for alloc in nc.m.functions[0].allocations:
        if not isinstance(alloc, mybir.MemoryLocationSet):
            continue
        name = alloc.memorylocations[0].name
        if alloc.kind == "ExternalInput":
            if name != partition_name:
                in_names.append(name)
        elif alloc.kind == "ExternalOutput":
            shape = tuple(alloc.tensor_shape)
            dtype = mybir.dt.np(alloc.dtype)
            out_names.append(name)
            out_avals.append(jax.core.ShapedArray(shape, dtype))
            zero_outs.append(np.zeros(shape, dtype))
    n_params = len(in_names)
    all_in_names = in_names + out_names
    if partition_name is not None:
        all_in_names = all_in_names + [partition_name]

    def _body(*args):
        operands = list(args)
        if partition_name is not None:
            operands.append(bass2jax.partition_id_tensor())
        outs = _bass_exec_p.bind(
            *operands,
            out_avals=tuple(out_avals),
            in_names=tuple(all_in_names),
            out_names=tuple(out_names),
            lowering_input_output_aliases=(),
            sim_require_finite=True,
            sim_require_nnan=True,
            nc=nc,
        )
        return tuple(outs)

    devices = jax.devices()[:NC]
    mesh = Mesh(np.asarray(devices), ("core",))
    n_outs = len(out_names)
    sharded = jax.jit(
        shard_map(_body, mesh=mesh,
                  in_specs=(PartitionSpec("core"),) * (n_params + n_outs),
                  out_specs=(PartitionSpec("core"),) * n_outs,
                  check_rep=False),
        keep_unused=True)

    entry["sharded"] = sharded

    def run(in_maps):
        concat_in = [np.concatenate([np.asarray(m[n]) for m in in_maps], axis=0)
                     for n in in_names]
        concat_zero = [np.zeros((NC * z.shape[0], *z.shape[1:]), z.dtype)
                       for z in zero_outs]
        out_arrs = sharded(*concat_in, *concat_zero)
        return [
            {n: np.asarray(out_arrs[i]).reshape(NC, *out_avals[i].shape)[c]
             for i, n in enumerate(out_names)}
            for c in range(NC)
        ]

    entry["runner"] = run
    return run


def kernel(**inputs):
    in_maps, flags = _prep_inputs(**inputs)
    entry = _get_compiled(flags)
    results = _get_runner(entry)(in_maps)
    return np.concatenate([r["out"] for r in results], axis=0)


# revision 25
# speedup vs baseline: 15329.1648x; 5.2994x over previous
"""Trainium2 Bass kernel for nn_DecoderRNN (LSTM decoder + vocab projection).

Sharding: batch 128 -> 16 per core across 8 cores (data parallel). LSTM
weights/embedding/fc replicated. Matmuls in fp16 with split-precision
corrections (fp32 PSUM accumulate):
  - W_hh = Whi + Wlo (both fp16), h = h_hi + h_lo (both fp16):
      gates ~= Whi@[h_hi|h_lo] + Wlo@h_hi   (drops only Wlo@h_lo ~ 2^-22)
  - W_ih split the same way (one-time precompute cost).
  - fc_W single fp16 (output-stage error does not compound).

Per-core layout: recurrent state kept transposed (D on partitions, batch on
free dim) so the h @ Whh^T matmuls chain without per-step transposes:
  gates^T[m-chunk 128, b] = WhhT_tile[k 128, m 128]^T @ hT[k 128, b]
Gate order host-permuted to [i, f, o, g] so sigmoid runs on one contiguous
block. Input-side gate preactivations (x_t @ Wih^T + biases) are precomputed
for all timesteps up front and spilled to DRAM, streamed back per step.
Logits per 128-token (t,b) chunk with H^T tiles stationary and fc_W^T
streaming; log-softmax without max-subtraction (h is layernormed so logits
are small), exp+sum via ACT accum_out.
"""

import numpy as np
import ml_dtypes  # noqa: F401

import concourse.bass as bass
import concourse.mybir as mybir
import concourse.tile as tile
from concourse import bacc
from concourse.masks import make_identity

F32 = mybir.dt.float32
F16 = mybir.dt.float16
I32 = mybir.dt.int32
AF = mybir.ActivationFunctionType
OP = mybir.AluOpType

NC = 8          # cores
B = 128         # global batch
BC = B // NC    # batch per core (16)
T = 33          # timesteps (1 feature + 32 caption)
LCAP = 32
D = 512
V = 10000
G = 4 * D       # 2048 gate dim
NK = D // 128   # 4 k-chunks
NM = G // 128   # 16 m-chunks
TOK = BC * T    # 528 tokens per core
NBT = BC * LCAP // 128  # 4 output (t,b) chunks of 128
NW = 20         # vocab chunks of 500
VW = V // NW    # 500
LN_EPS = 1e-5

f16 = np.float16


def _ap_insert_bcast(ap_obj, pos, count):
    """Insert a stride-0 dim of size `count` at free-dim position `pos`."""
    dims = [list(dd) for dd in ap_obj.ap]
    dims.insert(1 + pos, [0, count])
    return bass.AP(ap_obj.tensor, ap_obj.offset, dims)


def _emit(nc, tc, d, flags, bench_iters=0):
    """Emit the full per-core program. d: dict of dram tensors."""
    from contextlib import ExitStack

    has_fcb = flags["has_fcb"]
    has_ln = flags["has_ln"]

    es = ExitStack()
    cpool = es.enter_context(tc.tile_pool(name="const", bufs=1))
    ppool = es.enter_context(tc.tile_pool(name="persist", bufs=1))
    dpool = es.enter_context(tc.tile_pool(name="dram", bufs=1, space="DRAM"))

    ident = cpool.tile([128, 128], F16)
    make_identity(nc, ident[:])
    ones_col = cpool.tile([128, 1], F32)   # lhsT for partition sums
    nc.vector.memset(ones_col[:], 1.0)
    ones_row = cpool.tile([1, 128], F32)   # lhsT for broadcast matmul
    nc.vector.memset(ones_row[:], 1.0)
    eps_sb = cpool.tile([1, 1], F32)
    nc.vector.memset(eps_sb[:], LN_EPS)

    # ---- persistent tiles ----
    # whh_sb free layout: (l, hl, k, g)
    whh_sb = ppool.tile([128, 2 * 2 * NK * G], F16)   # 64KB/part
    for l in range(2):
        for hl in range(2):
            nc.sync.dma_start(
                whh_sb[:, (l * 2 + hl) * NK * G:(l * 2 + hl + 1) * NK * G]
                .rearrange("p (k g) -> p k g", k=NK),
                d["whhT"].ap()[l, hl].rearrange("(k p) g -> p k g", p=128))

    def whh_tile(l, hl, k, m):
        off = ((l * 2 + hl) * NK + k) * G + m * 128
        return whh_sb[:, off:off + 128]

    bias_sb = ppool.tile([128, 32], F32)           # (l, m)
    nc.sync.dma_start(bias_sb[:], d["biasT"].ap())
    fc_sb = ppool.tile([128, NK * V], F16)         # (k, v) 80KB/part
    nc.sync.dma_start(fc_sb[:].rearrange("p (k v) -> p k v", k=NK),
                      d["fcWT"].ap().rearrange("(k p) v -> p k v", p=128))
    if has_fcb:
        fcb_sb = ppool.tile([1, V], F16)
        nc.sync.dma_start(fcb_sb[:], d["fcb"].ap())
        ones_row16 = cpool.tile([1, 128], F16)
        nc.vector.memset(ones_row16[:], 1.0)
    if has_ln:
        lng_sb = ppool.tile([128, NK], F32)
        nc.sync.dma_start(lng_sb[:], d["lng"].ap())
        lnb_sb = ppool.tile([128, NK], F32)
        nc.sync.dma_start(lnb_sb[:], d["lnb"].ap())

    HT = ppool.tile([128, NK * BC * LCAP], F16)    # (k, bt) bt=(t-1)*16+b
    # state: (k, [hi(16) | lo(16)]) per k-chunk
    hT = ppool.tile([128, NK * 2 * BC], F16)
    h1T = ppool.tile([128, NK * 2 * BC], F16)
    cT = ppool.tile([128, NK * BC], F32)

    preT_d = dpool.tile([T, 128, 2 * NM * BC], F32)  # spill (t, p, (l m b))

    # ---- prologue: embedding gather + transpose into xT ----
    prolog_es = ExitStack()
    lpool = prolog_es.enter_context(tc.tile_pool(name="prolog", bufs=2))

    xT = lpool.tile([128, NK * TOK], F16, bufs=1)  # (k, tok) tok=b*33+t
    xT4 = xT[:].rearrange("p (k b s) -> p k b s", k=NK, b=BC)  # s = t 0..32

    cap_sb = lpool.tile([128, 4], I32, tag="cap")
    nc.sync.dma_start(cap_sb[:], d["cap"].ap())

    ftmp = lpool.tile([128, NK * BC], F16, tag="ftmp")
    nc.sync.dma_start(ftmp[:], d["featT"].ap())
    nc.vector.tensor_copy(xT4[:, :, :, 0],
                          ftmp[:].rearrange("p (k b) -> p k b", k=NK))

    with tc.tile_pool(name="ps_t", bufs=2, space="PSUM") as ps_t:
        for g in range(4):
            gt = lpool.tile([128, D], F16, tag="gather")
            nc.gpsimd.indirect_dma_start(
                out=gt[:], out_offset=None,
                in_=d["embW"].ap(),
                in_offset=bass.IndirectOffsetOnAxis(ap=cap_sb[:, g:g + 1], axis=0))
            for k in range(NK):
                tps = ps_t.tile([128, 128], F16, tag="tps")
                nc.tensor.transpose(tps[:], gt[:, k * 128:(k + 1) * 128], ident[:])
                # tokens e = g*128 + r*32 + s -> b = 4g+r, slot = b*33 + 1 + s
                nc.vector.tensor_copy(
                    xT4[:, k, 4 * g:4 * g + 4, 1:33],
                    tps[:].rearrange("p (r s) -> p r s", r=4))

    # ---- pre-gate precompute (all up front): jobs (l, m, t-half) ----
    t_halves = [(0, 17), (17, 16)]  # (t0, nt)
    with tc.tile_pool(name="ps_pre", bufs=2, space="PSUM") as lps:
        for l in range(2):
            wih_hi = lpool.tile([128, NK * G], F16, tag="wih_hi", name="wih_hi", bufs=1)
            wih_lo = lpool.tile([128, NK * G], F16, tag="wih_lo", name="wih_lo", bufs=1)
            for hl, wt in ((0, wih_hi), (1, wih_lo)):
                nc.sync.dma_start(
                    wt[:].rearrange("p (k g) -> p k g", k=NK),
                    d["wihT"].ap()[l, hl].rearrange("(k p) g -> p k g", p=128))
            for m in range(NM):
                for hh in range(2):
                    t0, nt = t_halves[hh]
                    n = BC * nt
                    pps = lps.tile([128, 272], F32, tag="pps")
                    for hl, wt in ((0, wih_hi), (1, wih_lo)):
                        for k in range(NK):
                            rhs = xT4[:, k, :, t0:t0 + nt].rearrange(
                                "p b s -> p s b")
                            nc.tensor.matmul(
                                pps[:, :n],
                                wt[:, (k * G + m * 128):(k * G + m * 128 + 128)],
                                rhs, start=(hl == 0 and k == 0),
                                stop=(hl == 1 and k == NK - 1))
                    sc = lpool.tile([128, 272], F32, tag="psc")
                    nc.scalar.activation(
                        sc[:, :n], pps[:, :n], AF.Identity,
                        bias=bias_sb[:, l * NM + m:l * NM + m + 1])
                    # DRAM (t, p, (l m b)); sc iter (t, b)
                    dst = preT_d[:].rearrange(
                        "s p (l m b) -> p l m s b", l=2, m=NM)[:, l, m,
                                                              t0:t0 + nt, :]
                    nc.sync.dma_start(
                        dst, sc[:, :n].rearrange("p (s b) -> p s b", b=BC))

    prolog_es.close()

    # ---- main pools ----
    mpool = es.enter_context(tc.tile_pool(name="main", bufs=3))
    psA = es.enter_context(tc.tile_pool(name="ps_gates", bufs=2, space="PSUM"))
    psL = es.enter_context(tc.tile_pool(name="ps_logit", bufs=2, space="PSUM"))
    psS = es.enter_context(tc.tile_pool(name="ps_small", bufs=1, space="PSUM"))

    # ---- logits machinery ----
    logit_state = {}   # q -> dict(logits tile, sums tile, nlz tile)

    def emit_exp_job(q, w):
        st = logit_state.get(q)
        if st is None:
            st = {
                "logits": mpool.tile([128, V], F16, tag="logits_q",
                                     name="logits_q", bufs=1),
                "sums": mpool.tile([128, NW], F32, tag="sums_q", name="sums_q"),
                "nlz": mpool.tile([128, 1], F32, tag="nlz_q", name="nlz_q"),
            }
            logit_state[q] = st
        lp = psL.tile([128, VW], F32, tag="lps")
        for k in range(NK):
            nc.tensor.matmul(
                lp[:],
                HT[:, k * 512 + q * 128: k * 512 + (q + 1) * 128],
                fc_sb[:, k * V + w * VW: k * V + (w + 1) * VW],
                start=(k == 0), stop=(k == NK - 1) and not has_fcb)
        if has_fcb:
            nc.tensor.matmul(lp[:], ones_row16[:],
                             fcb_sb[:, w * VW:(w + 1) * VW],
                             start=False, stop=True)
        nc.vector.tensor_copy(st["logits"][:, w * VW:(w + 1) * VW], lp[:])
        ex = mpool.tile([128, VW], F32, tag="exp_scratch", bufs=2)
        nc.scalar.activation(ex[:], lp[:], AF.Exp,
                             accum_out=st["sums"][:, w:w + 1])

    def emit_stats_job(q):
        st = logit_state[q]
        z = mpool.tile([128, 1], F32, tag="z_q")
        nc.vector.tensor_reduce(z[:], st["sums"][:], mybir.AxisListType.X, OP.add)
        lz = mpool.tile([128, 1], F32, tag="lz_q")
        nc.scalar.activation(lz[:], z[:], AF.Ln)
        nc.vector.tensor_scalar_mul(st["nlz"][:], lz[:], -1.0)

    def emit_out_job(q, w):
        st = logit_state[q]
        ob = mpool.tile([128, VW], F32, tag="out_sb")
        nc.vector.tensor_scalar_add(ob[:], st["logits"][:, w * VW:(w + 1) * VW],
                                    st["nlz"][:, 0:1])
        # out rows bt = (t-1)*16 + b ; chunk q covers t-1 in [8q, 8q+8)
        dst = d["out"].ap().rearrange("b (q s) v -> q s b v", q=NBT)
        nc.sync.dma_start(dst[q, :, :, w * VW:(w + 1) * VW], ob[:])

    jobs = []  # flat job list in required order per q
    for q in range(NBT):
        jobs += [("exp", q, w) for w in range(NW)]
        jobs += [("stats", q, None)]
        jobs += [("out", q, w) for w in range(NW)]
    job_ready_t = {q: 8 * (q + 1) for q in range(NBT)}
    job_idx = [0]

    def run_job(j):
        kind, q, w = j
        if kind == "exp":
            emit_exp_job(q, w)
        elif kind == "stats":
            emit_stats_job(q)
        else:
            emit_out_job(q, w)

    def emit_ready_jobs(t, max_pe_jobs):
        pe_emitted = 0
        while job_idx[0] < len(jobs):
            kind, q, w = jobs[job_idx[0]]
            if t < job_ready_t[q]:
                break
            if kind == "exp":
                if pe_emitted >= max_pe_jobs:
                    break
                pe_emitted += 1
            run_job(jobs[job_idx[0]])
            job_idx[0] += 1

    def write_split(dst_split, src_f32, tag):
        """dst_split layout [128, (NK, 2, BC)]: hi|lo fp16 halves of src."""
        d3 = dst_split[:].rearrange("p (k hl b) -> p k hl b", k=NK, hl=2)
        s3 = src_f32[:].rearrange("p (k b) -> p k b", k=NK)
        nc.vector.tensor_copy(d3[:, :, 0, :], s3)           # hi = fp16(x)
        back = mpool.tile([128, NK * BC], F32, tag=tag, name=tag)
        nc.vector.tensor_copy(back[:], d3[:, :, 0, :])      # hi back to f32
        nc.vector.tensor_tensor(d3[:, :, 1, :], s3,
                                back[:].rearrange("p (k b) -> p k b", k=NK),
                                OP.subtract)                 # lo = x - hi

    # ---- recurrence (optionally repeated for benchmarking) ----
    def main_body():
     nc.vector.memset(hT[:], 0.0)
     nc.vector.memset(cT[:], 0.0)
     for t in range(T):
        pre_sb = mpool.tile([128, 2 * NM * BC], F32, tag="pre")  # (l, m, b)
        nc.sync.dma_start(pre_sb[:], preT_d[t])

        h2 = None
        for l in range(2):
            rhs_t = hT if l == 0 else h1T
            # psum layout per m: [hi-products(16) | lo-products(16)]
            gps = psA.tile([128, NM * 2 * BC], F32, tag="g")
            for m in range(NM):
                reg32 = gps[:, m * 32:(m + 1) * 32]
                reg16 = gps[:, m * 32:m * 32 + 16]
                for k in range(NK):
                    # Whi @ [h_hi | h_lo]  (N=32)
                    nc.tensor.matmul(
                        reg32, whh_tile(l, 0, k, m),
                        rhs_t[:, k * 32:(k + 1) * 32],
                        start=(k == 0), stop=False, skip_group_check=True)
                for k in range(NK):
                    # Wlo @ h_hi  (N=16) accumulated onto hi half
                    nc.tensor.matmul(
                        reg16, whh_tile(l, 1, k, m),
                        rhs_t[:, k * 32:k * 32 + 16],
                        start=False, stop=(k == NK - 1), skip_group_check=True)
            # gates = (hi-half + pre) + lo-half (one PSUM operand per op)
            g3 = gps[:].rearrange("p (m hl b) -> p hl m b", m=NM, hl=2)
            gs = mpool.tile([128, NM * BC], F32, tag="gs")
            gs3 = gs[:].rearrange("p (m b) -> p m b", m=NM)
            pre3 = pre_sb[:, l * NM * BC:(l + 1) * NM * BC].rearrange(
                "p (m b) -> p m b", m=NM)
            nc.vector.tensor_tensor(gs3, g3[:, 0], pre3, OP.add)
            nc.vector.tensor_tensor(gs3, gs3, g3[:, 1], OP.add)
            # gate blocks (m-major): i 0:64, f 64:128, o 128:192, g 192:256
            sg = mpool.tile([128, 192], F32, tag="sg")
            nc.scalar.activation(sg[:], gs[:, 0:192], AF.Sigmoid)
            tg = mpool.tile([128, 64], F32, tag="tg")
            nc.scalar.activation(tg[:], gs[:, 192:256], AF.Tanh)
            t1 = mpool.tile([128, 64], F32, tag="t1")
            nc.vector.tensor_tensor(t1[:], sg[:, 0:64], tg[:], OP.mult)
            t2 = mpool.tile([128, 64], F32, tag="t2")
            nc.vector.tensor_tensor(t2[:], sg[:, 64:128], cT[:], OP.mult)
            nc.vector.tensor_tensor(cT[:], t1[:], t2[:], OP.add)
            tch = mpool.tile([128, 64], F32, tag="tch")
            nc.scalar.activation(tch[:], cT[:], AF.Tanh)
            hval = mpool.tile([128, 64], F32, tag="hval", name="hval")
            nc.vector.tensor_tensor(hval[:], sg[:, 128:192], tch[:], OP.mult)
            if l == 0:
                write_split(h1T, hval, "back1")
            else:
                h2 = hval

        # ---- layernorm over d (partitions x 4 chunks) ----
        sq = mpool.tile([128, 64], F32, tag="sq")
        nc.vector.tensor_tensor(sq[:], h2[:], h2[:], OP.mult)
        sps = psS.tile([1, 128], F32, tag="s")
        nc.tensor.matmul(sps[0:1, 0:64], ones_col[:], h2[:], start=True, stop=True)
        nc.tensor.matmul(sps[0:1, 64:128], ones_col[:], sq[:], start=True, stop=True)
        ssb = mpool.tile([1, 128], F32, tag="ssb")
        nc.vector.tensor_copy(ssb[:], sps[:])
        stat = mpool.tile([1, 64], F32, tag="stat")  # mu|rs|ms/var|mu2/sd
        nc.vector.tensor_reduce(stat[0:1, 0:16],
                                ssb[0:1, 0:64].rearrange("p (c b) -> p b c", c=NK),
                                mybir.AxisListType.X, OP.add)
        nc.vector.tensor_reduce(stat[0:1, 32:48],
                                ssb[0:1, 64:128].rearrange("p (c b) -> p b c", c=NK),
                                mybir.AxisListType.X, OP.add)
        nc.vector.tensor_scalar_mul(stat[0:1, 0:16], stat[0:1, 0:16], 1.0 / D)
        nc.vector.tensor_scalar_mul(stat[0:1, 32:48], stat[0:1, 32:48], 1.0 / D)
        nc.vector.tensor_tensor(stat[0:1, 48:64], stat[0:1, 0:16],
                                stat[0:1, 0:16], OP.mult)
        nc.vector.tensor_tensor(stat[0:1, 32:48], stat[0:1, 32:48],
                                stat[0:1, 48:64], OP.subtract)
        nc.scalar.activation(stat[0:1, 48:64], stat[0:1, 32:48], AF.Sqrt,
                             bias=eps_sb[0:1, 0:1])
        nc.vector.reciprocal(stat[0:1, 16:32], stat[0:1, 48:64])
        bps = psS.tile([128, 32], F32, tag="b")
        nc.tensor.matmul(bps[:], ones_row[:], stat[0:1, 0:32], start=True, stop=True)
        mu_bc = _ap_insert_bcast(bps[:, 0:16], 0, NK)    # [128, 4c(bcast), 16b]
        rs_bc = _ap_insert_bcast(bps[:, 16:32], 0, NK)
        d0 = mpool.tile([128, 64], F32, tag="d0")
        nc.vector.tensor_tensor(
            d0[:].rearrange("p (c b) -> p c b", c=NK),
            h2[:].rearrange("p (c b) -> p c b", c=NK), mu_bc, OP.subtract)
        hln = mpool.tile([128, 64], F32, tag="hln", name="hln")
        nc.vector.tensor_tensor(
            hln[:].rearrange("p (c b) -> p c b", c=NK),
            d0[:].rearrange("p (c b) -> p c b", c=NK), rs_bc, OP.mult)
        if has_ln:
            gg = _ap_insert_bcast(lng_sb[:], 1, BC)      # [128, 4c, 16b(bcast)]
            bb = _ap_insert_bcast(lnb_sb[:], 1, BC)
            hl2 = mpool.tile([128, 64], F32, tag="hl2")
            nc.vector.tensor_tensor(
                hl2[:].rearrange("p (c b) -> p c b", c=NK),
                hln[:].rearrange("p (c b) -> p c b", c=NK), gg, OP.mult)
            nc.vector.tensor_tensor(
                hln[:].rearrange("p (c b) -> p c b", c=NK),
                hl2[:].rearrange("p (c b) -> p c b", c=NK), bb, OP.add)
        write_split(hT, hln, "back0")
        if t > 0:
            nc.vector.tensor_copy(
                HT[:].rearrange("p (k n) -> p k n", k=NK)[:, :, (t - 1) * BC:t * BC],
                hT[:].rearrange("p (k hl b) -> p k hl b", k=NK, hl=2)[:, :, 0, :])

        emit_ready_jobs(t, max_pe_jobs=2)

     emit_ready_jobs(T, max_pe_jobs=10 ** 9)
     assert job_idx[0] == len(jobs)

    if bench_iters:
        with tc.For_i(0, bench_iters, 1):
            main_body()
    else:
        main_body()

    es.close()


def _build(flags, bench_iters=0):
    nc = bacc.Bacc("TRN2", target_bir_lowering=False, debug=False, num_devices=1)
    d = {}
    d["cap"] = nc.dram_tensor("cap", [128, 4], I32, kind="ExternalInput")
    d["featT"] = nc.dram_tensor("featT", [128, NK * BC], F16, kind="ExternalInput")
    d["embW"] = nc.dram_tensor("embW", [V, D], F16, kind="ExternalInput")
    d["wihT"] = nc.dram_tensor("wihT", [2, 2, D, G], F16, kind="ExternalInput")
    d["whhT"] = nc.dram_tensor("whhT", [2, 2, D, G], F16, kind="ExternalInput")
    d["biasT"] = nc.dram_tensor("biasT", [128, 32], F32, kind="ExternalInput")
    d["fcWT"] = nc.dram_tensor("fcWT", [D, V], F16, kind="ExternalInput")
    if flags["has_fcb"]:
        d["fcb"] = nc.dram_tensor("fcb", [1, V], F16, kind="ExternalInput")
    if flags["has_ln"]:
        d["lng"] = nc.dram_tensor("lng", [128, NK], F32, kind="ExternalInput")
        d["lnb"] = nc.dram_tensor("lnb", [128, NK], F32, kind="ExternalInput")
    d["out"] = nc.dram_tensor("out", [BC, LCAP, V], F32, kind="ExternalOutput")

    with tile.TileContext(nc) as tc:
        _emit(nc, tc, d, flags, bench_iters)
    nc.compile()
    return nc


def _split16(x):
    """x (f32) -> stacked [2, ...] fp16 hi/lo pair."""
    hi = x.astype(f16)
    lo = (x - hi.astype(np.float32)).astype(f16)
    return np.stack([hi, lo])


def _prep_inputs(features, caption, emb_W, W_ih, W_hh, b_ih, b_hh,
                 ln_g, ln_b, fc_W, fc_b):
    """Host-side marshaling: shard, permute gate order, transpose, cast."""
    features = np.asarray(features, np.float32)
    caption = np.asarray(caption)
    emb_W = np.asarray(emb_W, np.float32)
    W_ih = np.asarray(W_ih, np.float32)
    W_hh = np.asarray(W_hh, np.float32)
    b_sum = np.asarray(b_ih, np.float32) + np.asarray(b_hh, np.float32)
    ln_g = np.asarray(ln_g, np.float32)
    ln_b = np.asarray(ln_b, np.float32)
    fc_W = np.asarray(fc_W, np.float32)
    fc_b = np.asarray(fc_b, np.float32)

    # gate order [i,f,g,o] -> [i,f,o,g]
    perm = np.concatenate([np.arange(0, 2 * D), np.arange(3 * D, 4 * D),
                           np.arange(2 * D, 3 * D)])
    wihT = _split16(np.ascontiguousarray(
        W_ih[:, perm, :].transpose(0, 2, 1)))                # [2hl, 2l, D, G]
    wihT = np.ascontiguousarray(wihT.transpose(1, 0, 2, 3))  # [2l, 2hl, D, G]
    whhT = _split16(np.ascontiguousarray(
        W_hh[:, perm, :].transpose(0, 2, 1)))
    whhT = np.ascontiguousarray(whhT.transpose(1, 0, 2, 3))
    biasT = np.ascontiguousarray(
        b_sum[:, perm].reshape(2, NM, 128).transpose(2, 0, 1).reshape(128, 32))
    fcWT = np.ascontiguousarray(fc_W.T).astype(f16)          # [D, V]
    embW = emb_W.astype(f16)

    flags = {
        "has_fcb": bool(np.any(fc_b)),
        "has_ln": not (np.all(ln_g == 1.0) and np.all(ln_b == 0.0)),
    }

    in_maps = []
    for c in range(NC):
        bs = slice(c * BC, (c + 1) * BC)
        m = {
            "cap": np.ascontiguousarray(caption[bs]).reshape(4, 128).T
            .copy().astype(np.int32),
            "featT": np.ascontiguousarray(
                features[bs, 0, :].T.reshape(NK, 128, BC).transpose(1, 0, 2)
                .reshape(128, NK * BC)).astype(f16),
            "embW": embW,
            "wihT": wihT,
            "whhT": whhT,
            "biasT": biasT,
            "fcWT": fcWT,
        }
        if flags["has_fcb"]:
            m["fcb"] = fc_b.reshape(1, V).astype(f16)
        if flags["has_ln"]:
            m["lng"] = np.ascontiguousarray(ln_g.reshape(NK, 128).T)
            m["lnb"] = np.ascontiguousarray(ln_b.reshape(NK, 128).T)
        in_maps.append(m)
    return in_maps, flags


_CACHE = {}


def _get_compiled(flags, bench_iters=0):
    key = (tuple(sorted(flags.items())), bench_iters)
    if key not in _CACHE:
        _CACHE[key] = {"nc": _build(flags, bench_iters)}
    return _CACHE[key]


def _get_runner(entry):
    """Cached jitted 8-core SPMD runner (mirrors bass2jax.run_bass_via_pjrt,
    but reusable across calls so the NEFF compiles once)."""
    if "runner" in entry:
        return entry["runner"]
    import jax
    from jax.sharding import Mesh, PartitionSpec
    from jax.experimental.shard_map import shard_map
    from concourse import bass2jax
    from concourse.bass2jax import _bass_exec_p, install_neuronx_cc_hook

    nc = entry["nc"]
    install_neuronx_cc_hook()
    partition_name = (nc.partition_id_tensor.name
                      if nc.partition_id_tensor else None)
    in_names, out_names, out_avals, zero_outs = [], [], [], []
    for alloc in nc.m.functions[0].allocations:
        if not isinstance(alloc, mybir.MemoryLocationSet):
            continue
        name = alloc.memorylocations[0].name
        if alloc.kind == "ExternalInput":
            if name != partition_name:
                in_names.append(name)
        elif alloc.kind == "ExternalOutput":
            shape = tuple(alloc.tensor_shape)
            dtype = mybir.dt.np(alloc.dtype)
            out_names.append(name)
            out_avals.append(jax.core.ShapedArray(shape, dtype))
            zero_outs.append(np.zeros(shape, dtype))
    n_params = len(in_names)
    all_in_names = in_names + out_names
    if partition_name is not None:
        all_in_names = all_in_names + [partition_name]

    def _body(*args):
        operands = list(args)
        if partition_name is not None:
            operands.append(bass2jax.partition_id_tensor())
        outs = _bass_exec_p.bind(
            *operands,
            out_avals=tuple(out_avals),
            in_names=tuple(all_in_names),
            out_names=tuple(out_names),
            lowering_input_output_aliases=(),
            sim_require_finite=True,
            sim_require_nnan=True,
            nc=nc,
        )
        return tuple(outs)

    devices = jax.devices()[:NC]
    mesh = Mesh(np.asarray(devices), ("core",))
    n_outs = len(out_names)
    sharded = jax.jit(
        shard_map(_body, mesh=mesh,
                  in_specs=(PartitionSpec("core"),) * (n_params + n_outs),
                  out_specs=(PartitionSpec("core"),) * n_outs,
                  check_rep=False),
        keep_unused=True)

    entry["sharded"] = sharded

    def run(in_maps):
        concat_in = [np.concatenate([np.asarray(m[n]) for m in in_maps], axis=0)
                     for n in in_names]
        concat_zero = [np.zeros((NC * z.shape[0], *z.shape[1:]), z.dtype)
                       for z in zero_outs]
        out_arrs = sharded(*concat_in, *concat_zero)
        return [
            {n: np.asarray(out_arrs[i]).reshape(NC, *out_avals[i].shape)[c]
             for i, n in enumerate(out_names)}
            for c in range(NC)
        ]

    entry["runner"] = run
    return run


def kernel(**inputs):
    in_maps, flags = _prep_inputs(**inputs)
    entry = _get_compiled(flags)
    results = _get_runner(entry)(in_maps)
    return np.concatenate([r["out"] for r in results], axis=0)
